# revision 16
# baseline (speedup 1.0000x reference)
"""Trainium2 Bass kernel for nn_AttentionModel (RNN + attention loop + fc).

Full inputs in, full outputs out. Data-parallel over batch across 8 cores:
each core gets 32 batch elements, keeps its slice of the RNN hidden states
(out_pre) resident in SBUF in two bf16 layouts (n-major for the score einsum,
s-major for the attention einsum), and runs the 256-iteration sequential
attention loop entirely on-chip. No collectives.
"""

from contextlib import ExitStack

import numpy as np

import concourse.bass as bass
import concourse.mybir as mybir
import concourse.tile as tile
from concourse import bass_utils
from concourse.masks import make_identity

FP32 = mybir.dt.float32
BF16 = mybir.dt.bfloat16

# Full-problem dims (hardcoded per harness contract)
S_FULL, B_FULL, NI_FULL, N_FULL = 512, 256, 64, 256
N_CORES = 8


def split_multi_waits(nc):
    """Walrus in this toolchain rejects >1 semaphore wait per instruction.
    Split extra waits into standalone single-wait EventSemaphore ops on the
    same engine (the same thing raw-bass wait_ge() emits)."""
    n = 0
    for fn in nc.m.functions:
        for bb in fn.blocks:
            new = []
            for inst in bb.instructions:
                si = inst.sync_info
                if si is not None and len(si.on_wait) > 1:
                    waits = list(si.on_wait)
                    for w in waits[:-1]:
                        ev = mybir.InstEventSemaphore(
                            name=f"wsplit-{n}", engine=inst.engine,
                            sync_info=mybir.SyncInfo(on_wait=[w],
                                                     on_update=[]))
                        new.append(ev)
                        n += 1
                    si.on_wait = [waits[-1]]
                new.append(inst)
            bb.instructions = new
    return n


def build_nc(S=S_FULL, BL=B_FULL // N_CORES, NI=NI_FULL, N=N_FULL, iters=None,
             unroll=1):
    """Emit the single-core program. All cores run the same program on
    different batch slices.

    Wait-limit discipline: this walrus config allows only ONE semaphore wait
    per instruction, and Tile emits waits non-transitively. The kernel is
    structured so every instruction has at most one foreign-semaphore
    dependency: phase 1 writes hidden states straight from the Activation
    engine into GT (bf16), no pool is ever released (PSUM = exactly 8 banks
    live), and tiny observer ops advance an engine's vector clock where a
    second dependency would otherwise appear.
    """
    if iters is None:
        iters = N
    NC = N // 128   # n-chunks
    SC = S // 128   # s-chunks
    assert N % 128 == 0 and S % 128 == 0 and NI <= 64
    packed = S >= 256
    SH = S // 2 if packed else S

    nc = bass.Bass()

    xt = nc.declare_dram_parameter("xt", [128 if packed else NI, SH, BL], BF16,
                                   isOutput=False)
    wih = nc.declare_dram_parameter("wih", [2 * NI if packed else NI, N],
                                    BF16, isOutput=False)
    whh = nc.declare_dram_parameter("whh", [128, NC, N], BF16, isOutput=False)
    wcih = nc.declare_dram_parameter("wcih", [128, NC, N], FP32, isOutput=False)
    wchh = nc.declare_dram_parameter("wchh", [128, NC, N], FP32, isOutput=False)
    bias1 = nc.declare_dram_parameter("bias1", [128, NC], FP32, isOutput=False)
    biasc = nc.declare_dram_parameter("biasc", [128, NC], FP32, isOutput=False)
    wfc = nc.declare_dram_parameter("wfc", [128, NC], FP32, isOutput=False)
    bfc = nc.declare_dram_parameter("bfc", [1, 1], FP32, isOutput=False)
    y = nc.declare_dram_parameter("y", [1, BL], FP32, isOutput=True)

    with tile.TileContext(nc) as tc, \
            tc.tile_pool(name="persist", bufs=1) as persist, \
            tc.tile_pool(name="p1_x", bufs=1) as p1x, \
            tc.tile_pool(name="lsb", bufs=2) as ls, \
            tc.tile_pool(name="p1_psum", bufs=1, space="PSUM") as p1p, \
            tc.tile_pool(name="tr_psum", bufs=1, space="PSUM") as trp, \
            tc.tile_pool(name="dum", bufs=1, space="PSUM") as dum, \
            tc.tile_pool(name="l_psum", bufs=1, space="PSUM") as lp, \
            tc.tile_pool(name="l_psum2", bufs=1, space="PSUM") as lp2:
        # ---------------- persistent SBUF state ----------------
        GT = persist.tile([128, NC, BL, S], BF16)     # n-major out_pre
        G = persist.tile([128, SC, BL, N], BF16)      # s-major out_pre
        hpdiag = persist.tile([128, NC, BL, BL], BF16)
        pdiag = persist.tile([128, SC, BL, BL], BF16)
        hp = persist.tile([128, NC, BL], FP32)        # hidden_post, [n-part, b]
        attr = persist.tile([128, NC, BL], FP32)      # attention, [n-part, b]
        ident = persist.tile([128, 128], BF16)
        wih_sb = persist.tile([2 * NI if packed else NI, N], BF16)
        whh_sb = persist.tile([128, NC, N], BF16)
        wcih_sb = persist.tile([128, NC, N], FP32)
        wchh_sb = persist.tile([128, NC, N], FP32)
        bias1_sb = persist.tile([128, NC], FP32)
        biasc_sb = persist.tile([128, NC], FP32)
        wfc_sb = persist.tile([128, NC], FP32)
        bfc_sb = persist.tile([1, 1], FP32)
        scr_act = persist.tile([1, NC], FP32)         # ACT-tick relay
        scr_a = persist.tile([128, 2], FP32)          # ACT observer scratch
        scr_v = persist.tile([1, 1], FP32)            # DVE observer scratch

        def diag_dest(t, chunks, chunk_stride):
            base = t[:, :, :, :]
            return bass.AP(
                tensor=base.tensor,
                offset=base.offset,
                ap=[base.ap[0], [chunk_stride, chunks], [BL + 1, BL]],
            )

        # ---------------- setup ----------------
        nc.sync.dma_start(out=wih_sb, in_=wih[:])
        nc.sync.dma_start(out=whh_sb, in_=whh[:])
        nc.sync.dma_start(out=wcih_sb, in_=wcih[:])
        nc.sync.dma_start(out=wchh_sb, in_=wchh[:])
        nc.sync.dma_start(out=bias1_sb, in_=bias1[:])
        nc.sync.dma_start(out=biasc_sb, in_=biasc[:])
        nc.sync.dma_start(out=wfc_sb, in_=wfc[:])
        nc.sync.dma_start(out=bfc_sb, in_=bfc[:])
        nc.gpsimd.memset(ident, 0.0)
        nc.gpsimd.affine_select(
            out=ident, in_=ident,
            compare_op=mybir.AluOpType.not_equal, fill=1.0, base=0,
            pattern=[[-1, 128]], channel_multiplier=1)
        nc.vector.memset(hpdiag, 0.0)
        nc.vector.memset(pdiag, 0.0)
        nc.vector.memset(hp, 0.0)
        xt_sb = p1x.tile([128 if packed else NI, SH, BL], BF16)
        nc.sync.dma_start(out=xt_sb, in_=xt[:])

        # observers: every engine sees each setup semaphore once, so real
        # instructions later need at most one wait each
        dps = dum.tile([1, 32], FP32)
        obs = [xt_sb, wih_sb, whh_sb, hp, hpdiag, pdiag, ident]
        for i, tgt in enumerate(obs):
            sl = tgt[0:1, 0:1] if len(tgt.shape) == 2 else (
                tgt[0:1, 0, 0:1] if len(tgt.shape) == 3 else
                tgt[0:1, 0, 0, 0:1])
            nc.tensor.matmul(out=dps[0:1, i:i + 1], lhsT=sl, rhs=sl,
                             start=True, stop=True)
        # fp32 observer targets need an fp32 matmul (dtype must be uniform)
        obs32 = [wcih_sb, wchh_sb, wfc_sb]
        for j, tgt in enumerate(obs32):
            sl = tgt[0:1, 0:1] if len(tgt.shape) == 2 else tgt[0:1, 0, 0:1]
            nc.tensor.matmul(out=dps[0:1, 10 + j:11 + j], lhsT=sl, rhs=sl,
                             start=True, stop=True)
        nc.scalar.copy(out=scr_a[:, 0:1], in_=bias1_sb[:, 0:1])
        nc.scalar.copy(out=scr_a[:, 1:2], in_=biasc_sb[:, 0:1])
        nc.vector.tensor_copy(out=scr_v, in_=bfc_sb)
        tc.no_sync_barrier()

        # ---------------- phase 1: RNN recurrence (bf16) ----------------
        # tanh writes straight into GT; the next step reads its h from GT
        for t in range(S):
            if packed:
                pbase = 64 * (t // SH)
                x_rhs = xt_sb[pbase:pbase + NI, t % SH, :]
            else:
                pbase = 0
                x_rhs = xt_sb[:, t, :]
            ps = p1p.tile([128, NC, BL], FP32, tag="ps_h")
            for m in range(NC):
                nc.tensor.matmul(
                    out=ps[:, m, :],
                    lhsT=wih_sb[pbase:pbase + NI, m * 128:(m + 1) * 128],
                    rhs=x_rhs, start=True, stop=(t == 0))
                for k in range(NC):
                    if t == 0:
                        continue  # h0 = 0
                    nc.tensor.matmul(
                        out=ps[:, m, :],
                        lhsT=whh_sb[:, k, m * 128:(m + 1) * 128],
                        rhs=GT[:, k, :, t - 1],
                        start=False, stop=(k == NC - 1))
            for m in range(NC):
                nc.scalar.activation(
                    out=GT[:, m, :, t], in_=ps[:, m, :],
                    func=mybir.ActivationFunctionType.Tanh,
                    bias=bias1_sb[:, m:m + 1])
        # ACT-tick relay: reads both m-chunks of the last step
        nc.scalar.copy(out=scr_act, in_=GT[0:1, :, 0, S - 1])
        sa = scr_act[0:1, 0:1]
        nc.tensor.matmul(out=dps[0:1, 29:30], lhsT=sa, rhs=sa,
                         start=True, stop=True)
        tc.no_sync_barrier()

        # ---------------- transpose pass: GT -> G ----------------
        for b in range(BL):
            for cn in range(NC):
                for cs in range(SC):
                    pt = trp.tile([128, 128], BF16, tag="pt")
                    nc.tensor.transpose(
                        out=pt,
                        in_=GT[:, cn, b, cs * 128:(cs + 1) * 128],
                        identity=ident)
                    nc.vector.tensor_copy(
                        out=G[:, cs, b, cn * 128:(cn + 1) * 128], in_=pt)
        tc.no_sync_barrier()

        # ---------------- phase 2: attention loop ----------------
        def att_body():
            # scores: [BL, S] = per-b <GT_b, hp_b>, diag-weights PSUM trick
            ps_sc = lp.tile([BL, S], FP32, tag="ps_sc")
            first = True
            for b in range(BL):
                for k in range(NC):
                    nc.tensor.matmul(
                        out=ps_sc, lhsT=hpdiag[:, k, b, :],
                        rhs=GT[:, k, b, :], start=first,
                        stop=(b == BL - 1 and k == NC - 1))
                    first = False
            # softmax over free axis (s)
            nmx = ls.tile([BL, 1], FP32, tag="nmx")
            nc.vector.tensor_reduce(
                out=nmx, in_=ps_sc, axis=mybir.AxisListType.X,
                op=mybir.AluOpType.max, negate=True)
            # ACT observer: see nmx (DVE) so Exp itself needs only the PE wait
            nc.scalar.copy(out=scr_a[0:BL, 0:1], in_=nmx)
            e_sb = ls.tile([BL, S], BF16, tag="e_sb")
            den = ls.tile([BL, 1], FP32, tag="den")
            nc.scalar.activation(
                out=e_sb, in_=ps_sc,
                func=mybir.ActivationFunctionType.Exp,
                bias=nmx, accum_out=den)
            rinv = ls.tile([BL, 1], FP32, tag="rinv")
            nc.vector.reciprocal(out=rinv, in_=den)
            # p^T via PE transpose (e unnormalized; att scaled later)
            ps_p = lp2.tile([128, SC, BL], BF16, tag="ps_p")
            for cs in range(SC):
                nc.tensor.transpose(
                    out=ps_p[:, cs, :],
                    in_=e_sb[:, cs * 128:(cs + 1) * 128],
                    identity=ident[0:BL, 0:BL])
            nc.vector.tensor_copy(
                out=diag_dest(pdiag, SC, BL * BL), in_=ps_p)
            # attention: [BL, N] accumulate over (b, cs)
            ps_at = lp.tile([BL, N], FP32, tag="ps_at")
            first = True
            for b in range(BL):
                for cs in range(SC):
                    nc.tensor.matmul(
                        out=ps_at, lhsT=pdiag[:, cs, b, :],
                        rhs=G[:, cs, b, :], start=first,
                        stop=(b == BL - 1 and cs == SC - 1))
                    first = False
            at_sb = ls.tile([BL, N], BF16, tag="at_sb")
            nc.vector.tensor_scalar_mul(at_sb, ps_at, rinv)
            ps_att = lp2.tile([128, NC, BL], BF16, tag="ps_att")
            for cn in range(NC):
                nc.tensor.transpose(
                    out=ps_att[:, cn, :],
                    in_=at_sb[:, cn * 128:(cn + 1) * 128],
                    identity=ident[0:BL, 0:BL])
            nc.vector.tensor_copy(out=attr, in_=ps_att)
            # update: hp_new = tanh(Wc_ih^T-mm(hp) + Wc_hh^T-mm(att) + bc)
            ps_hp = lp.tile([128, NC, BL], FP32, tag="ps_hp")
            for m in range(NC):
                for k in range(NC):
                    nc.tensor.matmul(
                        out=ps_hp[:, m, :],
                        lhsT=wcih_sb[:, k, m * 128:(m + 1) * 128],
                        rhs=hp[:, k, :], start=(k == 0), stop=False)
                for k in range(NC):
                    nc.tensor.matmul(
                        out=ps_hp[:, m, :],
                        lhsT=wchh_sb[:, k, m * 128:(m + 1) * 128],
                        rhs=attr[:, k, :], start=False,
                        stop=(k == NC - 1))
            for m in range(NC):
                nc.scalar.activation(
                    out=hp[:, m, :], in_=ps_hp[:, m, :],
                    func=mybir.ActivationFunctionType.Tanh,
                    bias=biasc_sb[:, m:m + 1])
            nc.vector.tensor_copy(
                out=diag_dest(hpdiag, NC, BL * BL), in_=hp)

        if iters % unroll == 0 and iters // unroll > 1:
            with tc.For_i(0, iters // unroll, 1):
                for _ in range(unroll):
                    att_body()
        else:
            for _ in range(iters):
                att_body()

        # ---------------- fc head ----------------
        ps_y = lp.tile([1, BL], FP32, tag="ps_hp")
        for k in range(NC):
            nc.tensor.matmul(
                out=ps_y, lhsT=wfc_sb[:, k:k + 1], rhs=hp[:, k, :],
                start=(k == 0), stop=(k == NC - 1))
        y_sb = ls.tile([1, BL], FP32, tag="y_sb")
        nc.vector.tensor_scalar_add(y_sb, ps_y, bfc_sb[0:1, 0:1])
        nc.sync.dma_start(out=y[:], in_=y_sb)

    split_multi_waits(nc)
    return nc


def make_core_inputs(X, W_ih, W_hh, b_ih, b_hh, Wc_ih, Wc_hh, bc_ih, bc_hh,
                     W_fc, b_fc, core, n_cores=N_CORES):
    """Host-side layout prep for one core's batch slice."""
    S, B, NI = X.shape
    N = W_hh.shape[0]
    NC = N // 128
    BL = B // n_cores
    packed = S >= 256
    SH = S // 2 if packed else S
    import ml_dtypes
    Xc = np.ascontiguousarray(
        np.transpose(X[:, core * BL:(core + 1) * BL, :], (2, 0, 1))
    ).astype(ml_dtypes.bfloat16)  # [NI, S, BL]
    if packed:
        xt = np.concatenate([Xc[:, :SH, :], Xc[:, SH:, :]], axis=0)
    else:
        xt = Xc

    def chunked_T(W):  # W: [out, in] -> lhsT layout [128, NC, out]
        WT = np.ascontiguousarray(W.T.astype(np.float32))  # [in, out]
        return np.ascontiguousarray(
            WT.reshape(NC, 128, W.shape[0]).transpose(1, 0, 2))

    def perpart(v):  # [N] -> [128, NC]
        return np.ascontiguousarray(v.reshape(NC, 128).T.astype(np.float32))

    return {
        "xt": np.ascontiguousarray(xt),
        "wih": np.ascontiguousarray(
            np.concatenate([W_ih.T] * 2, axis=0).astype(ml_dtypes.bfloat16)
            if packed else W_ih.T.astype(ml_dtypes.bfloat16)),
        "whh": chunked_T(W_hh).astype(ml_dtypes.bfloat16),
        "wcih": chunked_T(Wc_ih),
        "wchh": chunked_T(Wc_hh),
        "bias1": perpart(b_ih + b_hh),
        "biasc": perpart(bc_ih + bc_hh),
        "wfc": perpart(W_fc[0]),
        "bfc": np.full((1, 1), np.float32(b_fc[0])),
    }


_NC_CACHE = {}


def kernel(X, W_ih, W_hh, b_ih, b_hh, Wc_ih, Wc_hh, bc_ih, bc_hh, W_fc, b_fc):
    args = (X, W_ih, W_hh, b_ih, b_hh, Wc_ih, Wc_hh, bc_ih, bc_hh, W_fc, b_fc)
    args = tuple(np.asarray(a, np.float32) for a in args)
    if "nc" not in _NC_CACHE:
        _NC_CACHE["nc"] = build_nc()
    nc = _NC_CACHE["nc"]
    in_maps = [make_core_inputs(*args, core=c) for c in range(N_CORES)]
    res = bass_utils.run_bass_kernel_spmd(nc, in_maps, list(range(N_CORES)))
    outs = [res.results[c]["y"].reshape(-1) for c in range(N_CORES)]
    return np.concatenate(outs).reshape(B_FULL, 1).astype(np.float32)


if __name__ == "__main__":
    import reference

    inp = {k: np.asarray(v) for k, v in reference.setup_inputs().items()}
    out = kernel(**inp)
    import jax.numpy as jnp

    ref = np.asarray(reference.reference(**{k: jnp.asarray(v)
                                            for k, v in inp.items()}))
    err = np.abs(out - ref)
    print("absmax err:", err.max(), "rel:", err.max() / np.abs(ref).max())
